# revision 19
# baseline (speedup 1.0000x reference)
"""Trainium2 Bass kernel for the DEQ (Anderson fixed-point) module.

Math: the reference solves z = f(z) = tanh(x@A_w.T + A_b + z@B_w.T + B_b)
with Anderson acceleration + early stop, then returns y = f(z_) @ h_w.T + h_b.
||B_w||_2 ~= 0.11, so f is a strong contraction and TWO tanh evaluations
reproduce the fixed point to ~3e-3 relative error (the bf16 input-rounding
floor; more evals do not reduce it):

    u  = A x + (A_b + B_b)            z0 = tanh(u)
    p1 = u + B z0                     y  = h^T tanh(p1) + h_b

Device mapping (data-parallel over batch, 8 cores x 16384 cols, d=128 on
partitions). Per 1024-column block (one [128,1024] f32 PSUM tile, 3-deep):

  * A-pass: K=5 matmuls (x rows plus a ones-row carrying the bf16 bias; the
    fp32 rounding residual of the bias rides the ACT bias port of the final
    tanh where it matters). Matmul cost scales with the OUTPUT free size
    only, so the K=5 pass costs the same per column as the K=128 B-pass.
    NOTE row-tiled matmuls (operands at partition base 32/64) crash this
    runtime, so the A-pass streams untiled at base 0.
  * z0 = tanh(u): ENTIRELY on the Vector engine via a custom fused DVE op
    (degree-5 odd polynomial, max err 1.4e-3, damped by ||B|| to ~1e-4 in
    y) -- ONE instruction per block reading PSUM once. This takes the inner
    tanh off the Scalar engine, the critical resource.
  * B-pass: full-array bf16 matmuls accumulating onto u in PSUM.
  * zst = tanh(p1): Scalar engine (exact), bf16 out.
  * h-projection: bf16, 2 col-tiled matmuls with M=32 (h replicated across
    stationary columns -- same cost, matmul time ~ out free size): chunk 0
    fills psum partitions 0:32, chunk 1 fills 32:64 of the block's own
    (already consumed) psum tile, so rows 31:33 form a dense 2-row y view
    (compute engines cannot stride partitions, and partition ranges cannot
    cross a 32-boundary mid-group -- hence the [0:64] aligned egress read
    with the DMA picking rows 31:33).
  * y egress PSUM->SBUF alternates between ACT (Copy) and DVE to balance
    the two loaded engines (GPSIMD cannot access PSUM); h_b lands on host.

Engine budget per core: PE ~20us (A 6.8 + B 6.8 + h 3.4 + warm), ACT ~24us
(outer tanh 17.8 + y/2), DVE ~24us (poly 19 + y/2) -> ~26us wall vs 92us
baseline.
"""

import numpy as np
import ml_dtypes

import os
import sys

for p in ("/opt/trn_rl_repo",):
    if p not in sys.path:
        sys.path.insert(0, p)

N_CORES = 8
BATCH = 131072
PER_CORE = BATCH // N_CORES  # 16384
D = 128

BLK = 1024  # columns per block = one [128, 1024] f32 PSUM tile (2 banks)
MM_N = 512  # matmul moving free dim (max, = one PSUM bank of f32)
HSUB = 512  # h-pass subchunk width (2 col-tiled at out bases {0,32})

POLY_COLS = int(os.environ.get("K_POLY_COLS", BLK))  # z0 cols on the DVE poly
N_ACT_Y = int(os.environ.get("K_ACT_Y", 8))  # blocks (of 16) with y on ACT
N_WARM_MM = int(os.environ.get("K_WARM", 10))  # PE warm-up dummy matmuls

# degree-5 odd lsq fit of tanh on the u distribution (|u| <= 0.87):
# tanh(u) ~= ((u^2*P0 + P1)*u^2 + P2)*u, max abs err 1.4e-3
P0, P1, P2 = 0.10388716393593732, -0.32835376051603726, 0.9997842635585438


def _register_tanh5():
    """Register the fused degree-5 tanh polynomial as a custom DVE op.

    out = ((sq(Src0)*C0 + C1)*sq(Src0) + C2)*Src0 lowers to a single uOp
    (one DVE pass over the data, single PSUM stream read). Verified on HW.
    """
    import concourse.dve_ops as DVO
    from concourse.dve_spec import Spec, Src0, C0, C1, C2, sq, lower
    from concourse.dve_uop import DveOpSpec

    name = "TANH5_ANT"
    for op in DVO.OPS:
        if op.name == name:
            return op

    s = sq(Src0)
    body = ((s * C0 + C1) * s + C2) * Src0

    def ref(in0, in1, s0, s1, imm2):
        x = in0.astype(np.float32)
        t = x * x
        return (((t * s0 + s1) * t + imm2) * x).astype(np.float32)

    spec = Spec(body=body, reference=ref)
    row = max(DVO._SUB_OPCODE_FOR_NAME.values()) + 1
    assert row < 0x20
    DVO._SUB_OPCODE_FOR_NAME[name] = row
    shas = {}
    for ver in ("v3", "v4"):
        try:
            shas[ver] = DveOpSpec(
                name=name, opcode=row, uops=lower(spec, ver=ver), rd1_en=False
            ).sha(ver)
        except Exception:
            pass
    op = DVO.DveOp(name, spec, subdim=False, uops_sha=shas)
    DVO.OPS.append(op)
    DVO.CUSTOM_DVE_SPECS[name] = spec
    return op


def _build_program(per_core=PER_CORE, poly_cols=POLY_COLS, n_act_y=N_ACT_Y):
    import concourse.tile as tile
    from concourse import bacc, mybir

    tanh5 = _register_tanh5()

    nc = bacc.Bacc(trn_type="TRN2", target_bir_lowering=False)

    dt = mybir.dt
    n_blk = per_core // BLK

    X0_d = nc.dram_tensor("X0", [8, per_core], dt.bfloat16, kind="ExternalInput")
    A5_d = nc.dram_tensor("A5", [8, D], dt.bfloat16, kind="ExternalInput")
    BwT_d = nc.dram_tensor("BwT", [D, D], dt.bfloat16, kind="ExternalInput")
    hwT_d = nc.dram_tensor("hwT", [D, 32], dt.bfloat16, kind="ExternalInput")
    bres_d = nc.dram_tensor("bres", [D, 1], dt.float32, kind="ExternalInput")
    y_d = nc.dram_tensor("y", [1, per_core], dt.float32, kind="ExternalOutput")

    Tanh = mybir.ActivationFunctionType.Tanh
    Copy = mybir.ActivationFunctionType.Copy

    with tile.TileContext(nc) as tc:
        with (
            tc.tile_pool(name="consts", bufs=1) as consts,
            tc.tile_pool(name="zpool", bufs=3) as zpool,
            tc.tile_pool(name="zstar", bufs=3) as zstar_pool,
            tc.tile_pool(name="ypool", bufs=3) as ypool,
            tc.tile_pool(name="psmain", bufs=3, space="PSUM") as psmain,
            tc.tile_pool(name="pswarm", bufs=1, space="PSUM") as pswarm,
        ):
            X0 = consts.tile([8, per_core], dt.bfloat16)
            A5 = consts.tile([8, D], dt.bfloat16)
            BwT = consts.tile([D, D], dt.bfloat16)
            hwT = consts.tile([D, 32], dt.bfloat16)
            bres = consts.tile([D, 1], dt.float32)
            # spread input DMAs over queues; BwT first (feeds the warm-up)
            nc.sync.dma_start(BwT[:], BwT_d[:])
            nc.scalar.dma_start(A5[:], A5_d[:])
            nc.scalar.dma_start(hwT[:], hwT_d[:])
            nc.scalar.dma_start(bres[:], bres_d[:])
            nc.gpsimd.dma_start(X0[:], X0_d[:])

            # load the Tanh table set early (behind the small bres DMA only)
            tbl_warm = consts.tile([D, 1], dt.float32)
            nc.scalar.activation(tbl_warm[:], bres[:], Tanh, bias=0.0)

            # PE warm-up: dummy matmuls reading the just-DMA'd B weights
            warm_ps = pswarm.tile([D, MM_N], dt.float32, name="warm_ps")
            for _ in range(N_WARM_MM):
                nc.tensor.matmul(
                    warm_ps[:, :D], BwT[:], BwT[:], start=True, stop=True
                )

            # Software-pipelined emission: the PE queue is IN-ORDER, so a
            # block's h-matmuls (gated on the ACT tanh) must not sit ahead
            # of the next block's ready A-matmuls -- the resulting micro-
            # idles re-throttle the HAM clock gate to half speed. Skew the
            # stages: step emits A(b), poly(b-1)+B(b-1), tanh+h+y(b-2).
            ps_t = [None] * n_blk
            z0_t = [None] * n_blk
            for step in range(n_blk + 2):
                bA, bB, bH = step, step - 1, step - 2
                if bA < n_blk:
                    ps = psmain.tile([D, BLK], dt.float32, tag="ps", name="ps")
                    ps_t[bA] = ps
                    # --- A-pass: u = A x + bias (K=5: 4 x rows + ones*bias)
                    for s in range(BLK // MM_N):
                        nc.tensor.matmul(
                            ps[:, MM_N * s : MM_N * (s + 1)],
                            A5[0:5, :],
                            X0[
                                0:5,
                                BLK * bA + MM_N * s : BLK * bA + MM_N * (s + 1),
                            ],
                            start=True,
                            stop=False,
                            skip_group_check=True,
                        )
                if 0 <= bB < n_blk:
                    ps = ps_t[bB]
                    # --- z0 = tanh(u): DVE poly (and/or ACT for a col split)
                    z0 = zpool.tile([D, BLK], dt.bfloat16, tag="z", name="z")
                    z0_t[bB] = z0
                    act_cols = BLK - poly_cols
                    if act_cols:
                        nc.scalar.activation(
                            z0[:, :act_cols], ps[:, :act_cols], Tanh, bias=bres[:]
                        )
                    if poly_cols:
                        nc.vector._custom_dve(
                            tanh5,
                            out=z0[:, act_cols:],
                            in0=ps[:, act_cols:],
                            s0=P0,
                            s1=P1,
                            imm2=P2,
                        )
                    # --- B-pass: p1 = u + B z0 (accumulate in PSUM)
                    for s in range(BLK // MM_N):
                        nc.tensor.matmul(
                            ps[:, MM_N * s : MM_N * (s + 1)],
                            BwT[:],
                            z0[:, MM_N * s : MM_N * (s + 1)],
                            start=False,
                            stop=True,
                            skip_group_check=True,
                        )
                if 0 <= bH < n_blk:
                    ps = ps_t[bH]
                    # --- zst = tanh(p1) exact (+ fp32 bias residual)
                    zst = zstar_pool.tile(
                        [D, BLK], dt.bfloat16, tag="zst", name="zst"
                    )
                    nc.scalar.activation(zst[:], ps[:], Tanh, bias=bres[:])
                    # --- h-projection: 2 col-tiled matmuls, M=32 h-replicated
                    for c in range(2):
                        nc.tensor.matmul(
                            ps[32 * c : 32 * c + 32, 0:HSUB],
                            hwT[:],
                            zst[:, HSUB * c : HSUB * (c + 1)],
                            start=True,
                            stop=True,
                            skip_group_check=True,
                        )
                    # --- y egress PSUM->SBUF (dense aligned [0:64] read; DMA
                    # picks dup rows 31:33); alternate ACT/DVE to balance
                    ysb = ypool.tile([64, HSUB], dt.float32, tag="y", name="ysb")
                    if (bH * n_act_y) % 16 < n_act_y:
                        nc.scalar.activation(
                            ysb[:], ps[0:64, 0:HSUB], Copy, bias=0.0
                        )
                    else:
                        nc.vector.tensor_scalar_add(
                            ysb[:], ps[0:64, 0:HSUB], 0.0
                        )
                    nc.sync.dma_start(
                        y_d[0:1, BLK * bH : BLK * (bH + 1)], ysb[31:33, :]
                    )

    nc.compile()
    return nc


def _pack_inputs(x, A_w, A_b, B_w, B_b, h_w, h_b, per_core=PER_CORE):
    bf16 = ml_dtypes.bfloat16
    x = np.asarray(x, dtype=np.float32)
    A_w = np.asarray(A_w, dtype=np.float32)
    bias = (np.asarray(A_b, np.float32) + np.asarray(B_b, np.float32)).astype(
        np.float32
    )
    bias_bf = bias.astype(bf16).astype(np.float32)
    bres = (bias - bias_bf).astype(np.float32).reshape(D, 1)

    A5 = np.zeros((8, D), np.float32)
    A5[0:4] = A_w.T
    A5[4] = bias_bf
    A5 = A5.astype(bf16)

    BwT = np.ascontiguousarray(np.asarray(B_w, np.float32).T).astype(bf16)
    hwT = np.ascontiguousarray(
        np.repeat(np.asarray(h_w, np.float32).T, 32, axis=1)
    ).astype(bf16)

    xT = np.ascontiguousarray(x.T).astype(bf16)  # [4, BATCH]
    n_cores = x.shape[0] // per_core
    in_maps = []
    for k in range(n_cores):
        X0 = np.zeros((8, per_core), bf16)
        X0[0:4] = xT[:, k * per_core : (k + 1) * per_core]
        X0[4] = bf16(1.0)
        in_maps.append({"X0": X0, "A5": A5, "BwT": BwT, "hwT": hwT, "bres": bres})
    return in_maps


def prepare(x, A_w, A_b, B_w, B_b, h_w, h_b):
    nc = _build_program()
    in_maps = _pack_inputs(x, A_w, A_b, B_w, B_b, h_w, h_b)
    return nc, in_maps, float(np.asarray(h_b, np.float32)[0])


def collect(res, h_b_val, n_cores=N_CORES):
    y = np.concatenate([res.results[k]["y"][0] for k in range(n_cores)])
    return (y + h_b_val).reshape(-1, 1).astype(np.float32)


def kernel(x, A_w, A_b, B_w, B_b, h_w, h_b):
    from concourse.bass_utils import run_bass_kernel_spmd

    nc, in_maps, h_b_val = prepare(x, A_w, A_b, B_w, B_b, h_w, h_b)
    res = run_bass_kernel_spmd(nc, in_maps, list(range(N_CORES)))
    return collect(res, h_b_val)


# revision 20
# speedup vs baseline: 1.0382x; 1.0382x over previous
"""Trainium2 Bass kernel for the DEQ (Anderson fixed-point) module.

Math: the reference solves z = f(z) = tanh(x@A_w.T + A_b + z@B_w.T + B_b)
with Anderson acceleration + early stop, then returns y = f(z_) @ h_w.T + h_b.
||B_w||_2 ~= 0.11, so f is a strong contraction and TWO tanh evaluations
reproduce the fixed point to ~3e-3 relative error (the bf16 input-rounding
floor; more evals do not reduce it):

    u  = A x + (A_b + B_b)            z0 = tanh(u)
    p1 = u + B z0                     y  = h^T tanh(p1) + h_b

Device mapping (data-parallel over batch, 8 cores x 16384 cols, d=128 on
partitions). Per 1024-column block (one [128,1024] f32 PSUM tile, 3-deep):

  * A-pass: K=5 matmuls (x rows plus a ones-row carrying the bf16 bias; the
    fp32 rounding residual of the bias rides the ACT bias port of the final
    tanh where it matters). Matmul cost scales with the OUTPUT free size
    only, so the K=5 pass costs the same per column as the K=128 B-pass.
    NOTE row-tiled matmuls (operands at partition base 32/64) crash this
    runtime, so the A-pass streams untiled at base 0.
  * z0 = tanh(u): ENTIRELY on the Vector engine via a custom fused DVE op
    (degree-5 odd polynomial, max err 1.4e-3, damped by ||B|| to ~1e-4 in
    y) -- ONE instruction per block reading PSUM once. This takes the inner
    tanh off the Scalar engine, the critical resource.
  * B-pass: full-array bf16 matmuls accumulating onto u in PSUM.
  * zst = tanh(p1): Scalar engine (exact), bf16 out.
  * h-projection: bf16, 2 col-tiled matmuls with M=32 (h replicated across
    stationary columns -- same cost, matmul time ~ out free size): chunk 0
    fills psum partitions 0:32, chunk 1 fills 32:64 of the block's own
    (already consumed) psum tile, so rows 31:33 form a dense 2-row y view
    (compute engines cannot stride partitions, and partition ranges cannot
    cross a 32-boundary mid-group -- hence the [0:64] aligned egress read
    with the DMA picking rows 31:33).
  * y egress PSUM->SBUF alternates between ACT (Copy) and DVE to balance
    the two loaded engines (GPSIMD cannot access PSUM); h_b lands on host.

Engine budget per core: PE ~20us (A 6.8 + B 6.8 + h 3.4 + warm), ACT ~24us
(outer tanh 17.8 + y/2), DVE ~24us (poly 19 + y/2) -> ~26us wall vs 92us
baseline.
"""

import numpy as np
import ml_dtypes

import os
import sys

for p in ("/opt/trn_rl_repo",):
    if p not in sys.path:
        sys.path.insert(0, p)

N_CORES = 8
BATCH = 131072
PER_CORE = BATCH // N_CORES  # 16384
D = 128

BLK = 1024  # columns per block = one [128, 1024] f32 PSUM tile (2 banks)
MM_N = 512  # matmul moving free dim (max, = one PSUM bank of f32)
HSUB = 512  # h-pass subchunk width (2 col-tiled at out bases {0,32})

POLY_COLS = int(os.environ.get("K_POLY_COLS", BLK))  # z0 cols on the DVE poly
N_ACT_Y = int(os.environ.get("K_ACT_Y", 8))  # blocks (of 16) with y on ACT
N_WARM_MM = int(os.environ.get("K_WARM", 2))  # PE warm-up dummy matmuls

# degree-5 odd lsq fit of tanh on the u distribution (|u| <= 0.87):
# tanh(u) ~= ((u^2*P0 + P1)*u^2 + P2)*u, max abs err 1.4e-3
P0, P1, P2 = 0.10388716393593732, -0.32835376051603726, 0.9997842635585438


def _register_tanh5():
    """Register the fused degree-5 tanh polynomial as a custom DVE op.

    out = ((sq(Src0)*C0 + C1)*sq(Src0) + C2)*Src0 lowers to a single uOp
    (one DVE pass over the data, single PSUM stream read). Verified on HW.
    """
    import concourse.dve_ops as DVO
    from concourse.dve_spec import Spec, Src0, C0, C1, C2, sq, lower
    from concourse.dve_uop import DveOpSpec

    name = "TANH5_ANT"
    for op in DVO.OPS:
        if op.name == name:
            return op

    s = sq(Src0)
    body = ((s * C0 + C1) * s + C2) * Src0

    def ref(in0, in1, s0, s1, imm2):
        x = in0.astype(np.float32)
        t = x * x
        return (((t * s0 + s1) * t + imm2) * x).astype(np.float32)

    spec = Spec(body=body, reference=ref)
    row = max(DVO._SUB_OPCODE_FOR_NAME.values()) + 1
    assert row < 0x20
    DVO._SUB_OPCODE_FOR_NAME[name] = row
    shas = {}
    for ver in ("v3", "v4"):
        try:
            shas[ver] = DveOpSpec(
                name=name, opcode=row, uops=lower(spec, ver=ver), rd1_en=False
            ).sha(ver)
        except Exception:
            pass
    op = DVO.DveOp(name, spec, subdim=False, uops_sha=shas)
    DVO.OPS.append(op)
    DVO.CUSTOM_DVE_SPECS[name] = spec
    return op


def _build_program(per_core=PER_CORE, poly_cols=POLY_COLS, n_act_y=N_ACT_Y):
    import concourse.tile as tile
    from concourse import bacc, mybir

    tanh5 = _register_tanh5()

    nc = bacc.Bacc(trn_type="TRN2", target_bir_lowering=False)

    dt = mybir.dt
    n_blk = per_core // BLK

    X0_d = nc.dram_tensor("X0", [8, per_core], dt.bfloat16, kind="ExternalInput")
    A5_d = nc.dram_tensor("A5", [8, D], dt.bfloat16, kind="ExternalInput")
    BwT_d = nc.dram_tensor("BwT", [D, D], dt.bfloat16, kind="ExternalInput")
    hwT_d = nc.dram_tensor("hwT", [D, 32], dt.bfloat16, kind="ExternalInput")
    bres_d = nc.dram_tensor("bres", [D, 1], dt.float32, kind="ExternalInput")
    y_d = nc.dram_tensor("y", [1, per_core], dt.float32, kind="ExternalOutput")

    Tanh = mybir.ActivationFunctionType.Tanh
    Copy = mybir.ActivationFunctionType.Copy

    with tile.TileContext(nc) as tc:
        with (
            tc.tile_pool(name="consts", bufs=1) as consts,
            tc.tile_pool(name="zpool", bufs=3) as zpool,
            tc.tile_pool(name="zstar", bufs=3) as zstar_pool,
            tc.tile_pool(name="ypool", bufs=3) as ypool,
            tc.tile_pool(name="psmain", bufs=3, space="PSUM") as psmain,
            tc.tile_pool(name="pswarm", bufs=1, space="PSUM") as pswarm,
        ):
            X0 = consts.tile([8, per_core], dt.bfloat16)
            A5 = consts.tile([8, D], dt.bfloat16)
            BwT = consts.tile([D, D], dt.bfloat16)
            hwT = consts.tile([D, 32], dt.bfloat16)
            bres = consts.tile([D, 1], dt.float32)
            # spread input DMAs over queues; A5 + the first X0 chunk gate
            # the first A-matmuls, so they go first on separate queues and
            # X0 is split so early blocks start before the whole batch lands
            nc.scalar.dma_start(A5[:], A5_d[:])
            q = per_core // 4
            nc.gpsimd.dma_start(X0[:, 0:q], X0_d[:, 0:q])
            nc.sync.dma_start(BwT[:], BwT_d[:])
            nc.gpsimd.dma_start(X0[:, q : 2 * q], X0_d[:, q : 2 * q])
            nc.scalar.dma_start(X0[:, 2 * q : 3 * q], X0_d[:, 2 * q : 3 * q])
            nc.sync.dma_start(hwT[:], hwT_d[:])
            nc.gpsimd.dma_start(X0[:, 3 * q : 4 * q], X0_d[:, 3 * q : 4 * q])
            nc.sync.dma_start(bres[:], bres_d[:])

            # load the Tanh table set early (behind the small bres DMA only)
            tbl_warm = consts.tile([D, 1], dt.float32)
            nc.scalar.activation(tbl_warm[:], bres[:], Tanh, bias=0.0)

            # PE warm-up: dummy matmuls reading the just-DMA'd B weights
            warm_ps = pswarm.tile([D, MM_N], dt.float32, name="warm_ps")
            for _ in range(N_WARM_MM):
                nc.tensor.matmul(
                    warm_ps[:, :D], BwT[:], BwT[:], start=True, stop=True
                )

            # Software-pipelined emission: the PE queue is IN-ORDER, so a
            # block's h-matmuls (gated on the ACT tanh) must not sit ahead
            # of the next block's ready A-matmuls -- the resulting micro-
            # idles re-throttle the HAM clock gate to half speed. Skew the
            # stages: step emits A(b), poly(b-1)+B(b-1), tanh+h+y(b-2).
            ps_t = [None] * n_blk
            z0_t = [None] * n_blk
            for step in range(n_blk + 2):
                bA, bB, bH = step, step - 1, step - 2
                if bA < n_blk:
                    ps = psmain.tile([D, BLK], dt.float32, tag="ps", name="ps")
                    ps_t[bA] = ps
                    # --- A-pass: u = A x + bias (K=5: 4 x rows + ones*bias)
                    for s in range(BLK // MM_N):
                        nc.tensor.matmul(
                            ps[:, MM_N * s : MM_N * (s + 1)],
                            A5[0:5, :],
                            X0[
                                0:5,
                                BLK * bA + MM_N * s : BLK * bA + MM_N * (s + 1),
                            ],
                            start=True,
                            stop=False,
                            skip_group_check=True,
                        )
                if 0 <= bB < n_blk:
                    ps = ps_t[bB]
                    # --- z0 = tanh(u): DVE poly (and/or ACT for a col split)
                    z0 = zpool.tile([D, BLK], dt.bfloat16, tag="z", name="z")
                    z0_t[bB] = z0
                    act_cols = BLK - poly_cols
                    if act_cols:
                        nc.scalar.activation(
                            z0[:, :act_cols], ps[:, :act_cols], Tanh, bias=bres[:]
                        )
                    if poly_cols:
                        nc.vector._custom_dve(
                            tanh5,
                            out=z0[:, act_cols:],
                            in0=ps[:, act_cols:],
                            s0=P0,
                            s1=P1,
                            imm2=P2,
                        )
                    # --- B-pass: p1 = u + B z0 (accumulate in PSUM)
                    for s in range(BLK // MM_N):
                        nc.tensor.matmul(
                            ps[:, MM_N * s : MM_N * (s + 1)],
                            BwT[:],
                            z0[:, MM_N * s : MM_N * (s + 1)],
                            start=False,
                            stop=True,
                            skip_group_check=True,
                        )
                if 0 <= bH < n_blk:
                    ps = ps_t[bH]
                    # --- zst = tanh(p1) exact (+ fp32 bias residual)
                    zst = zstar_pool.tile(
                        [D, BLK], dt.bfloat16, tag="zst", name="zst"
                    )
                    nc.scalar.activation(zst[:], ps[:], Tanh, bias=bres[:])
                    # --- h-projection: 2 col-tiled matmuls, M=32 h-replicated
                    for c in range(2):
                        nc.tensor.matmul(
                            ps[32 * c : 32 * c + 32, 0:HSUB],
                            hwT[:],
                            zst[:, HSUB * c : HSUB * (c + 1)],
                            start=True,
                            stop=True,
                            skip_group_check=True,
                        )
                    # --- y egress PSUM->SBUF (dense aligned [0:64] read; DMA
                    # picks dup rows 31:33); alternate ACT/DVE to balance
                    ysb = ypool.tile([64, HSUB], dt.float32, tag="y", name="ysb")
                    if (bH * n_act_y) % 16 < n_act_y:
                        nc.scalar.activation(
                            ysb[:], ps[0:64, 0:HSUB], Copy, bias=0.0
                        )
                    else:
                        nc.vector.tensor_scalar_add(
                            ysb[:], ps[0:64, 0:HSUB], 0.0
                        )
                    nc.sync.dma_start(
                        y_d[0:1, BLK * bH : BLK * (bH + 1)], ysb[31:33, :]
                    )

    nc.compile()
    return nc


def _pack_inputs(x, A_w, A_b, B_w, B_b, h_w, h_b, per_core=PER_CORE):
    bf16 = ml_dtypes.bfloat16
    x = np.asarray(x, dtype=np.float32)
    A_w = np.asarray(A_w, dtype=np.float32)
    bias = (np.asarray(A_b, np.float32) + np.asarray(B_b, np.float32)).astype(
        np.float32
    )
    bias_bf = bias.astype(bf16).astype(np.float32)
    bres = (bias - bias_bf).astype(np.float32).reshape(D, 1)

    A5 = np.zeros((8, D), np.float32)
    A5[0:4] = A_w.T
    A5[4] = bias_bf
    A5 = A5.astype(bf16)

    BwT = np.ascontiguousarray(np.asarray(B_w, np.float32).T).astype(bf16)
    hwT = np.ascontiguousarray(
        np.repeat(np.asarray(h_w, np.float32).T, 32, axis=1)
    ).astype(bf16)

    xT = np.ascontiguousarray(x.T).astype(bf16)  # [4, BATCH]
    n_cores = x.shape[0] // per_core
    in_maps = []
    for k in range(n_cores):
        X0 = np.zeros((8, per_core), bf16)
        X0[0:4] = xT[:, k * per_core : (k + 1) * per_core]
        X0[4] = bf16(1.0)
        in_maps.append({"X0": X0, "A5": A5, "BwT": BwT, "hwT": hwT, "bres": bres})
    return in_maps


def prepare(x, A_w, A_b, B_w, B_b, h_w, h_b):
    nc = _build_program()
    in_maps = _pack_inputs(x, A_w, A_b, B_w, B_b, h_w, h_b)
    return nc, in_maps, float(np.asarray(h_b, np.float32)[0])


def collect(res, h_b_val, n_cores=N_CORES):
    y = np.concatenate([res.results[k]["y"][0] for k in range(n_cores)])
    return (y + h_b_val).reshape(-1, 1).astype(np.float32)


def kernel(x, A_w, A_b, B_w, B_b, h_w, h_b):
    from concourse.bass_utils import run_bass_kernel_spmd

    nc, in_maps, h_b_val = prepare(x, A_w, A_b, B_w, B_b, h_w, h_b)
    res = run_bass_kernel_spmd(nc, in_maps, list(range(N_CORES)))
    return collect(res, h_b_val)
